# revision 1
# baseline (speedup 1.0000x reference)
"""DoubleWindowAttention kernel for 8 trn2 NeuronCores.

Strategy: data-parallel over batch B=32 -> 8 shards of 4. Window attention is
fully independent per (batch, window); the small conv weights and the
relative-position-bias table are replicated to every core.

The per-shard computation is expressed in JAX and compiled by neuronx-cc for
the Trainium2 cores (jax.pmap over the 8 axon-tunneled NeuronCores).
All shapes/constants of the problem are hardcoded (B,C,H,W = 32,256,64,64,
WS=8, SHIFT=4, HEADS=8).
"""

import functools

import jax
import jax.numpy as jnp
import numpy as np

B, C, H, W = 32, 256, 64, 64
WS, SHIFT, HEADS = 8, 4, 8
DK, L = C // HEADS, WS * WS
NW = (H // WS) * (W // WS)
NCORES = 8


def _rel_index():
    coords = np.stack(
        np.meshgrid(np.arange(WS), np.arange(WS), indexing="ij")
    ).reshape(2, -1)
    rel = (coords[:, :, None] - coords[:, None, :]).transpose(1, 2, 0)
    rel[..., 0] += WS - 1
    rel[..., 1] += WS - 1
    rel[..., 0] *= 2 * WS - 1
    return rel.sum(-1)  # (L, L) int


def _attn_mask():
    img = np.zeros((H, W))
    sl = [slice(0, -WS), slice(-WS, -SHIFT), slice(-SHIFT, None)]
    cnt = 0
    for hs in sl:
        for ws_ in sl:
            img[hs, ws_] = cnt
            cnt += 1
    mw = img.reshape(H // WS, WS, W // WS, WS).transpose(0, 2, 1, 3).reshape(-1, L)
    diff = mw[:, None, :] - mw[:, :, None]
    return np.where(diff != 0, -100.0, 0.0).astype(np.float32)  # (NW, L, L)


REL_IDX = _rel_index()
MASK_NP = _attn_mask()


def _conv1x1(w, b, t):
    # t: (b, C, H, W); w: (O, C)
    bsz = t.shape[0]
    tf = t.reshape(bsz, t.shape[1], H * W)
    out = jnp.einsum("oc,bcp->bop", w, tf, preferred_element_type=jnp.float32)
    return (out + b[None, :, None]).reshape(bsz, w.shape[0], H, W)


def _shift(t, s):
    a, b2 = jnp.split(t, 2, axis=1)
    return jnp.concatenate([a, jnp.roll(b2, (s, s), axis=(2, 3))], axis=1)


def _win_part(t):
    b = t.shape[0]
    t = t.reshape(b, C, H // WS, WS, W // WS, WS).transpose(0, 2, 4, 1, 3, 5)
    return t.reshape(-1, HEADS, DK, L)


def _win_rev(t):
    t = t.reshape(-1, H // WS, W // WS, C, WS, WS).transpose(0, 3, 1, 4, 2, 5)
    return t.reshape(-1, C, H, W)


def _shard_fn(x, wq, bq, wkv, bkv, wproj, bproj, rpb, mask):
    # x: (B/8, C, H, W)
    q = _conv1x1(wq, bq, x)
    kv = _conv1x1(wkv, bkv, x)
    k, v = kv[:, :C], kv[:, C:]
    q, k, v = _shift(q, -SHIFT), _shift(k, -SHIFT), _shift(v, -SHIFT)
    q, k, v = _win_part(q), _win_part(k), _win_part(v)
    attn = jnp.einsum("bhct,bhcs->bhts", q, k, preferred_element_type=jnp.float32) * (
        DK ** -0.5
    )
    bias = rpb[REL_IDX.reshape(-1)].reshape(L, L, HEADS).transpose(2, 0, 1)
    attn = attn + bias[None]
    attn = attn.reshape(-1, NW, HEADS, L, L) + mask[None, :, None]
    attn = jax.nn.softmax(attn.reshape(-1, HEADS, L, L), axis=-1)
    out = jnp.einsum("bhts,bhcs->bhct", attn, v, preferred_element_type=jnp.float32)
    out = _win_rev(out)
    out = _shift(out, SHIFT)
    return _conv1x1(wproj, bproj, out)


_pmapped = None


def _get_pmapped():
    global _pmapped
    if _pmapped is None:
        _pmapped = jax.pmap(_shard_fn, axis_name="b", devices=jax.devices()[:NCORES])
    return _pmapped


def kernel(x, wq, bq, wkv, bkv, wproj, bproj, rpb):
    x = np.asarray(x, dtype=np.float32)
    xs = x.reshape(NCORES, B // NCORES, C, H, W)

    def rep(a):
        a = np.asarray(a, dtype=np.float32)
        return np.broadcast_to(a, (NCORES,) + a.shape)

    fn = _get_pmapped()
    out = fn(
        xs,
        rep(np.asarray(wq)),
        rep(np.asarray(bq)),
        rep(np.asarray(wkv)),
        rep(np.asarray(bkv)),
        rep(np.asarray(wproj)),
        rep(np.asarray(bproj)),
        rep(np.asarray(rpb)),
        rep(MASK_NP),
    )
    out = np.asarray(out)
    return out.reshape(B, C, H, W)


# revision 2
# speedup vs baseline: 96.4122x; 96.4122x over previous
"""DoubleWindowAttention kernel for 8 trn2 NeuronCores.

Sharding: data-parallel over batch B=32 -> 8 shards of 4 batches. Window
attention is independent per (batch, window); conv weights and the
relative-position-bias table are replicated.

Per-shard compute runs on the Trainium2 cores via jax.pmap (XLA/neuronx-cc).
Device-side graph is kept PE-friendly:
  - 1x1 convs as single [256,256] x [256, 4*4096] matmuls
  - bias+mask table gathered on HOST (tiny) and passed in, so no device gather
  - matmul operands cast to bf16 (PE full rate; fp32 runs at 1/4), fp32 accum
All problem shapes are hardcoded (B,C,H,W = 32,256,64,64, WS=8, SHIFT=4, H=8).
"""

import jax
import jax.numpy as jnp
import numpy as np

B, C, H, W = 32, 256, 64, 64
WS, SHIFT, HEADS = 8, 4, 8
DK, L = C // HEADS, WS * WS
NW = (H // WS) * (W // WS)
NCORES = 8
BS = B // NCORES  # batches per core


def _rel_index():
    coords = np.stack(
        np.meshgrid(np.arange(WS), np.arange(WS), indexing="ij")
    ).reshape(2, -1)
    rel = (coords[:, :, None] - coords[:, None, :]).transpose(1, 2, 0)
    rel[..., 0] += WS - 1
    rel[..., 1] += WS - 1
    rel[..., 0] *= 2 * WS - 1
    return rel.sum(-1)  # (L, L) int


def _attn_mask():
    img = np.zeros((H, W))
    sl = [slice(0, -WS), slice(-WS, -SHIFT), slice(-SHIFT, None)]
    cnt = 0
    for hs in sl:
        for ws_ in sl:
            img[hs, ws_] = cnt
            cnt += 1
    mw = img.reshape(H // WS, WS, W // WS, WS).transpose(0, 2, 1, 3).reshape(-1, L)
    diff = mw[:, None, :] - mw[:, :, None]
    return np.where(diff != 0, -100.0, 0.0).astype(np.float32)  # (NW, L, L)


REL_IDX = _rel_index()
MASK_NP = _attn_mask()


def _roll2(t, s):
    # cyclic roll by s on the last two axes (H, W), via concat (XLA-friendly)
    t = jnp.concatenate([t[..., -s % H :, :], t[..., : -s % H, :]], axis=-2)
    t = jnp.concatenate([t[..., -s % W :], t[..., : -s % W]], axis=-1)
    return t


def _shift(t, s):
    a, b2 = jnp.split(t, 2, axis=1)
    return jnp.concatenate([a, _roll2(b2, s)], axis=1)


def _win_part(t):
    b = t.shape[0]
    t = t.reshape(b, C, H // WS, WS, W // WS, WS).transpose(0, 2, 4, 1, 3, 5)
    return t.reshape(-1, HEADS, DK, L)


def _win_rev(t):
    t = t.reshape(-1, H // WS, W // WS, C, WS, WS).transpose(0, 3, 1, 4, 2, 5)
    return t.reshape(-1, C, H, W)


def _shard_fn(x, wq, bq, wkv, bkv, wproj, bproj, biasmask):
    # x: (BS, C, H, W); biasmask: (NW, HEADS, L, L) = host-gathered rpb + mask
    bf = jnp.bfloat16
    xf = x.transpose(1, 0, 2, 3).reshape(C, BS * H * W)

    def conv(w, b):
        out = jnp.einsum(
            "oc,cp->op", w.astype(bf), xf.astype(bf),
            preferred_element_type=jnp.float32,
        ) + b[:, None]
        return out.reshape(w.shape[0], BS, H, W).transpose(1, 0, 2, 3)

    q = conv(wq, bq)
    kv = conv(wkv, bkv)
    k, v = kv[:, :C], kv[:, C:]
    q, k, v = _shift(q, -SHIFT), _shift(k, -SHIFT), _shift(v, -SHIFT)
    q, k, v = _win_part(q), _win_part(k), _win_part(v)  # (BS*NW, HEADS, DK, L)
    attn = jnp.einsum(
        "bhct,bhcs->bhts", q.astype(bf), k.astype(bf),
        preferred_element_type=jnp.float32,
    ) * (DK ** -0.5)
    attn = attn.reshape(BS, NW, HEADS, L, L) + biasmask[None]
    attn = jax.nn.softmax(attn.reshape(-1, HEADS, L, L), axis=-1)
    out = jnp.einsum(
        "bhts,bhcs->bhct", attn.astype(bf), v.astype(bf),
        preferred_element_type=jnp.float32,
    )
    out = _win_rev(out)
    out = _shift(out, SHIFT)
    outf = out.transpose(1, 0, 2, 3).reshape(C, BS * H * W)
    proj = jnp.einsum(
        "oc,cp->op", wproj.astype(bf), outf.astype(bf),
        preferred_element_type=jnp.float32,
    ) + bproj[:, None]
    return proj.reshape(C, BS, H, W).transpose(1, 0, 2, 3)


_pmapped = None


def _get_pmapped():
    global _pmapped
    if _pmapped is None:
        _pmapped = jax.pmap(_shard_fn, devices=jax.devices()[:NCORES])
    return _pmapped


def kernel(x, wq, bq, wkv, bkv, wproj, bproj, rpb):
    x = np.asarray(x, dtype=np.float32)
    xs = x.reshape(NCORES, BS, C, H, W)

    # host-side: gather rpb with the constant REL_IDX and fold in the window mask
    rpb = np.asarray(rpb, dtype=np.float32)
    bias = rpb[REL_IDX.reshape(-1)].reshape(L, L, HEADS).transpose(2, 0, 1)  # (H,L,L)
    biasmask = (bias[None] + MASK_NP[:, None]).astype(np.float32)  # (NW,HEADS,L,L)

    def rep(a):
        a = np.asarray(a, dtype=np.float32)
        return np.broadcast_to(a, (NCORES,) + a.shape)

    fn = _get_pmapped()
    out = fn(
        xs,
        rep(np.asarray(wq)),
        rep(np.asarray(bq)),
        rep(np.asarray(wkv)),
        rep(np.asarray(bkv)),
        rep(np.asarray(wproj)),
        rep(np.asarray(bproj)),
        rep(biasmask),
    )
    return np.asarray(out).reshape(B, C, H, W)
